# revision 57
# baseline (speedup 1.0000x reference)
"""Trainium2 Bass kernel for nn_Attention_52166672777669 (sparse_attention).

Math (reference):
    q  = LN(qx; g_q, b_q) @ wq.T                        # [256, 512]
    k  = LN(kx; g_k, b_k) @ wk.T                        # [256, 512, 512]
    S[q, kb, n] = (q[q] . k[kb, n]) / sqrt(512)         # masked, softmax over n
    out[q, kb, :] = sum_n P[q, kb, n] * kx[kb, n, :]    # [256, 256, 512]

Algebraic restructuring (exact up to fp rounding):
  S.T[n,q] = <kx[n]*rk_n, Qg[:,q]> with Qg = gk*(wk.T @ q_vec)/sqrt(C),
  column-centered over c (handles the LN mean term exactly since
  sum_c (kx[n,c]-m_n) = 0) and rk_n = rsqrt(var_n + eps).
  All q-side work and the k-side row stats are host-precomputed; rk and
  the padding mask fold into the packed kxt operand (masked columns
  zeroed).  Masked keys then get P = exp(0) = 1, neutralized by zeroed
  kxn rows (numerator) and a 0/1 validity column (denominator).

  Fully-masked 128-key tiles are skipped entirely: batches are sorted by
  valid-tile count and dealt round-robin to the 8 cores, so one static
  per-slot schedule (max count within each rank-8 window) serves all
  cores; skipped tiles contribute exactly zero.

Device inner loop per slot (cj valid n-tiles): 4*cj QK matmuls, cj Exps
on ACT, 2*cj AV + 2*cj denominator matmuls (denominator second so its
redundant LDWEIGHTS hides under the 512-col AV matmul), 2 reciprocal +
2 normalize on DVE.  3 DMAs: loads on sync queue, store on gpsimd queue.

Sharding: Bk split across 8 cores (32 key-batches each). No collectives.
"""

import os
import sys

import numpy as np

for _p in ("/opt/trn_rl_repo",):
    if _p not in sys.path and os.path.isdir(_p):
        sys.path.insert(0, _p)

Bq, Bk, Nk, C = 256, 256, 512, 512
NCORES = 8
BKPC = Bk // NCORES  # key-batch slots per core
EPS = 1e-5
NT = Nk // 128  # 4 n tiles per key batch
CT = C // 128   # 4 c tiles
QT = Bq // 128  # 2 query tiles

_cache = {}


def _schedule_from_mask(mask):
    """Sort batches by valid-tile count asc (small slots first: faster
    pipeline fill), deal round-robin to cores.

    Returns (perm [Bk], schedule [BKPC]) where core i's slot j processes
    original batch perm[j*NCORES + i] using schedule[j] n-tiles."""
    lengths = Nk - np.asarray(mask).sum(axis=1)          # valid keys per batch
    counts = np.ceil(lengths / 128).astype(np.int64)     # needed n-tiles
    perm = np.argsort(counts, kind="stable")
    schedule = [int(counts[perm[j * NCORES + NCORES - 1]]) for j in range(BKPC)]
    # interleave small/large slots: a small slot's DMA (~2.9us) matches its
    # PE time (~3.0us), so runs of small slots never build DMA lookahead and
    # the AV phase stalls on its kxn load.  Pairing each small slot with a
    # large one (PE 3.95us vs DMA 3.6us) keeps the prefetch ahead.
    half = BKPC // 2
    order = []
    for i in range(half):
        order += [i, half + i]
    order += list(range(2 * half, BKPC))
    schedule = [schedule[j] for j in order]
    perm = perm.reshape(BKPC, NCORES)[order].reshape(-1)
    return perm, schedule


def _build_nc(schedule):
    from contextlib import ExitStack

    import concourse.bacc as bacc
    import concourse.bass as bass
    import concourse.mybir as mybir
    import concourse.tile as tile

    f16 = mybir.dt.float16
    f32 = mybir.dt.float32
    AF = mybir.ActivationFunctionType
    ALU = mybir.AluOpType

    nc = bacc.Bacc()

    # kxt block t: cols [t*512 + ci*128 + dn] (t-major so slot loads truncate)
    kxt_d = nc.declare_dram_parameter("kxt", [BKPC, 128, NT * C], f16, isOutput=False)
    qg_d = nc.declare_dram_parameter("qg", [128, CT * Bq], f16, isOutput=False)
    kxn_d = nc.declare_dram_parameter("kxn", [BKPC, 128, NT * C], f16, isOutput=False)
    # 0/1 validity columns for all slots: col j*NT + t
    vb_d = nc.declare_dram_parameter("vb", [128, BKPC * NT], f16, isOutput=False)
    # packed output: [b][p][mt][c] — host unpacks to [b, mt*128+p, c]
    out_d = nc.declare_dram_parameter("out", [BKPC, 128, QT * C], f16, isOutput=True)

    with tile.TileContext(nc) as tc, ExitStack() as ctx:
        consts = ctx.enter_context(tc.tile_pool(name="consts", bufs=1))
        work = ctx.enter_context(tc.tile_pool(name="work", bufs=2))
        ps = ctx.enter_context(tc.tile_pool(name="ps", bufs=1, space="PSUM"))

        kxts = {}
        kxns = {}

        def load_slot(j, split=False):
            cj = schedule[j]
            kxt = work.tile([128, NT * C], f16, tag="kxt", bufs=8)
            if split:
                # per-tile loads so the first QK can start after one tile
                for t in range(cj):
                    nc.sync.dma_start(
                        kxt[:, t * C : (t + 1) * C], kxt_d[j, :, t * C : (t + 1) * C]
                    )
            else:
                nc.sync.dma_start(kxt[:, 0 : cj * C], kxt_d[j, :, 0 : cj * C])
            kxn = work.tile([128, NT * C], f16, tag="kxn", bufs=8)
            nc.sync.dma_start(kxn[:, 0 : cj * C], kxn_d[j, :, 0 : cj * C])
            kxts[j] = kxt
            kxns[j] = kxn

        # qg first (small, unblocks the PE), then ramp the slot pipeline;
        # vb is only needed by the first AV phase, so it loads after slot 0
        qg_all = consts.tile([128, CT * Bq], f16)
        nc.sync.dma_start(qg_all[:], qg_d[:, :])
        qgT = [qg_all[:, ci * Bq : (ci + 1) * Bq] for ci in range(CT)]

        # single ACT LUT load for the whole kernel: one dummy Exp up front
        dummy = work.tile([128, 1], f16, tag="dummy")
        nc.vector.memset(dummy[:], 0.0)
        nc.scalar.activation(dummy[:], dummy[:], AF.Exp, scale=0.0)

        # PE warm-up: scratch matmuls during the initial DMA fill ramp the
        # tensor-engine p-state so slot 0 runs at full clock
        warm = work.tile([128, Bq], f16, tag="warm")
        nc.vector.memset(warm[:], 0.0)
        for _ in range(20):
            pw = ps.tile([128, Bq], f32, tag="psa", bufs=4)
            nc.tensor.matmul(pw[:], warm[:, 0:128], warm[:], start=True, stop=True)

        load_slot(0, split=True)
        vb = consts.tile([128, BKPC * NT], f16)
        nc.sync.dma_start(vb[:], vb_d[:, :])
        load_slot(1, split=True)
        for j in range(2, 7):
            load_slot(j)

        for j in range(BKPC):
            cj = schedule[j]
            kxt = kxts.pop(j)
            kxn = kxns.pop(j)
            if j + 7 < BKPC:
                load_slot(j + 7)

            # scores S.T[n, q] per valid n-tile; exp -> pT fp16
            pT = []
            for t in range(cj):
                pa = ps.tile([128, Bq], f32, tag="psa", bufs=4)
                for ci in range(CT):
                    nc.tensor.matmul(
                        pa[:],
                        kxt[:, t * C + ci * 128 : t * C + (ci + 1) * 128],
                        qgT[ci],
                        start=(ci == 0),
                        stop=(ci == CT - 1),
                    )
                pe = work.tile([128, Bq], f16, tag=f"pT{t}", bufs=3)
                nc.scalar.activation(pe[:], pa[:], AF.Exp)
                pT.append(pe)

            # AV + denominator (denominator second: its LDW hides under AV).
            # One 2-col reciprocal after both AV groups frees pd in a single
            # early DVE op (no 900ns normalize mul queued ahead of it), so
            # the next slot's denominator never WAR-stalls on pd.
            osb = work.tile([128, QT * C], f16, tag="osb", bufs=4)
            pd = ps.tile([128, QT], f32, tag="psd", bufs=1)
            rd = work.tile([128, QT], f32, tag="rd", bufs=2)
            pos = []
            for mt in range(QT):
                po = ps.tile([128, C], f32, tag="pso", bufs=3)
                for t in range(cj):
                    lhs = pT[t][:, mt * 128 : (mt + 1) * 128]
                    nc.tensor.matmul(
                        po[:],
                        lhs,
                        kxn[:, t * C : (t + 1) * C],
                        start=(t == 0),
                        stop=(t == cj - 1),
                    )
                    nc.tensor.matmul(
                        pd[:, mt : mt + 1],
                        lhs,
                        vb[:, j * NT + t : j * NT + t + 1],
                        start=(t == 0),
                        stop=(t == cj - 1),
                    )
                pos.append(po)
            nc.vector.reciprocal(rd[:, 0:QT], pd[:, 0:QT])
            # normalize halves on two engines concurrently (Copy is
            # table-free on ACT, so the Exp LUT stays loaded)
            nc.vector.tensor_scalar(
                osb[:, 0:C], pos[0][:], rd[:, 0:1], None, op0=ALU.mult
            )
            nc.scalar.mul(osb[:, C : 2 * C], pos[1][:], rd[:, 1:2])
            if j == BKPC - 1:
                # split the final store so each half leaves as soon as ready
                nc.gpsimd.dma_start(out_d[j, :, 0:C], osb[:, 0:C])
                nc.gpsimd.dma_start(out_d[j, :, C : 2 * C], osb[:, C : 2 * C])
            else:
                nc.gpsimd.dma_start(out_d[j, :, :], osb[:])

    nc.compile()
    return nc


def _prep_host(qx, kx, key_padding_mask, ln_q_g, ln_q_b, ln_k_g, ln_k_b, wq, wk):
    f32 = np.float32

    # ---- q-side: Qg[c, q] fully host-computed (fp32), column-centered ----
    qx32 = np.asarray(qx, f32).reshape(Bq, C)
    m = qx32.mean(-1, keepdims=True)
    v = ((qx32 - m) ** 2).mean(-1, keepdims=True)
    lnq = (qx32 - m) / np.sqrt(v + EPS)
    lnq = lnq * np.asarray(ln_q_g, f32)[None, :] + np.asarray(ln_q_b, f32)[None, :]
    qvec = lnq @ np.asarray(wq, f32).T                      # [Bq, C]
    y = qvec @ np.asarray(wk, f32)                          # [Bq, C]
    G = (y * np.asarray(ln_k_g, f32)[None, :]) * (C ** -0.5)
    G = G - G.mean(axis=1, keepdims=True)                   # center over c
    Qg = np.ascontiguousarray(G.T)                          # [c, q]

    qg_pk = np.zeros((128, CT * Bq), np.float16)
    for ci in range(CT):
        qg_pk[:, ci * Bq : (ci + 1) * Bq] = Qg[ci * 128 : (ci + 1) * 128, :]

    # ---- k-side row stats (host): rk = rsqrt(var + eps), mask folded ----
    kx32 = np.asarray(kx, f32)                              # [Bk, Nk, C]
    km = kx32.mean(-1, keepdims=True)
    kv = ((kx32 - km) ** 2).mean(-1, keepdims=True)
    rk = 1.0 / np.sqrt(kv + EPS)                            # [Bk, Nk, 1]
    mask = np.asarray(key_padding_mask)                     # [Bk, Nk] True=pad
    valid = (~mask).astype(f32)[:, :, None]                 # [Bk, Nk, 1]

    perm, schedule = _schedule_from_mask(mask)

    kxt_full = kx32 * rk * valid                            # [Bk, Nk, C]
    kxn_full = np.asarray(kx, np.float16) * valid.astype(np.float16)

    in_maps = []
    for i in range(NCORES):
        bidx = perm[np.arange(BKPC) * NCORES + i]           # original batch ids
        kxt_s = kxt_full[bidx]                              # [BKPC, Nk, C] f32
        # block t holds all c for n-tile t, c-partition major:
        # kxt[b, p, t*C + ci*128+dn] = kx[b, t*128+dn, ci*128+p]
        kxt_pk = (
            kxt_s.reshape(BKPC, NT, 128, CT, 128)           # [b, t, dn, ci, p]
            .transpose(0, 4, 1, 3, 2)                       # [b, p, t, ci, dn]
            .reshape(BKPC, 128, NT * C)
        )
        kxt_send = np.ascontiguousarray(kxt_pk).astype(np.float16)

        kxn_s = kxn_full[bidx]                              # [BKPC, Nk, C] f16
        kxn_pk = (
            kxn_s.reshape(BKPC, NT, 128, C).transpose(0, 2, 1, 3).reshape(BKPC, 128, NT * C)
        )
        # validity blob: col j*NT + t = valid for keys t*128+p of slot j
        vr = valid[bidx, :, 0].reshape(BKPC, NT, 128).transpose(2, 0, 1)  # [p, b, t]
        vb_pk = np.ascontiguousarray(vr.reshape(128, BKPC * NT)).astype(np.float16)
        in_maps.append(
            dict(
                qg=qg_pk,
                kxt=kxt_send,
                kxn=np.ascontiguousarray(kxn_pk),
                vb=vb_pk,
            )
        )
    return in_maps, perm, schedule


def _get_nc(schedule):
    key = ("nc", tuple(schedule))
    if key not in _cache:
        _cache[key] = _build_nc(schedule)
    return _cache[key]


def kernel(**inputs) -> np.ndarray:
    from concourse.bass_utils import run_bass_kernel_spmd

    in_maps, perm, schedule = _prep_host(**inputs)
    nc = _get_nc(schedule)
    res = run_bass_kernel_spmd(nc, in_maps, list(range(NCORES)))
    full = np.empty((Bq, Bk, C), np.float16)
    for i in range(NCORES):
        o = res.results[i]["out"]  # [BKPC, 128, 2C] packed
        o = o.reshape(BKPC, 128, QT, C).transpose(0, 2, 1, 3).reshape(BKPC, Bq, C)
        bidx = perm[np.arange(BKPC) * NCORES + i]
        full[:, bidx, :] = o.transpose(1, 0, 2)
    return np.ascontiguousarray(full)


# revision 58
# speedup vs baseline: 1.1737x; 1.1737x over previous
"""Trainium2 Bass kernel for nn_Attention_52166672777669 (sparse_attention).

Math (reference):
    q  = LN(qx; g_q, b_q) @ wq.T                        # [256, 512]
    k  = LN(kx; g_k, b_k) @ wk.T                        # [256, 512, 512]
    S[q, kb, n] = (q[q] . k[kb, n]) / sqrt(512)         # masked, softmax over n
    out[q, kb, :] = sum_n P[q, kb, n] * kx[kb, n, :]    # [256, 256, 512]

Algebraic restructuring (exact up to fp rounding):
  S.T[n,q] = <kx[n]*rk_n, Qg[:,q]> with Qg = gk*(wk.T @ q_vec)/sqrt(C),
  column-centered over c (handles the LN mean term exactly since
  sum_c (kx[n,c]-m_n) = 0) and rk_n = rsqrt(var_n + eps).
  All q-side work and the k-side row stats are host-precomputed; rk and
  the padding mask fold into the packed kxt operand (masked columns
  zeroed).  Masked keys then get P = exp(0) = 1, neutralized by zeroed
  kxn rows (numerator) and a 0/1 validity column (denominator).

  Fully-masked 128-key tiles are skipped entirely: batches are sorted by
  valid-tile count and dealt round-robin to the 8 cores, so one static
  per-slot schedule (max count within each rank-8 window) serves all
  cores; skipped tiles contribute exactly zero.

Device inner loop per slot (cj valid n-tiles): 4*cj QK matmuls, cj Exps
on ACT, 2*cj AV + 2*cj denominator matmuls (denominator second so its
redundant LDWEIGHTS hides under the 512-col AV matmul), 2 reciprocal +
2 normalize on DVE.  3 DMAs: loads on sync queue, store on gpsimd queue.

Sharding: Bk split across 8 cores (32 key-batches each). No collectives.
"""

import os
import sys

import numpy as np

for _p in ("/opt/trn_rl_repo",):
    if _p not in sys.path and os.path.isdir(_p):
        sys.path.insert(0, _p)

Bq, Bk, Nk, C = 256, 256, 512, 512
NCORES = 8
BKPC = Bk // NCORES  # key-batch slots per core
EPS = 1e-5
NT = Nk // 128  # 4 n tiles per key batch
CT = C // 128   # 4 c tiles
QT = Bq // 128  # 2 query tiles

_cache = {}


def _schedule_from_mask(mask):
    """Sort batches by valid-tile count asc (small slots first: faster
    pipeline fill), deal round-robin to cores.

    Returns (perm [Bk], schedule [BKPC]) where core i's slot j processes
    original batch perm[j*NCORES + i] using schedule[j] n-tiles."""
    lengths = Nk - np.asarray(mask).sum(axis=1)          # valid keys per batch
    counts = np.ceil(lengths / 128).astype(np.int64)     # needed n-tiles
    perm = np.argsort(counts, kind="stable")
    schedule = [int(counts[perm[j * NCORES + NCORES - 1]]) for j in range(BKPC)]
    # interleave small/large slots: a small slot's DMA (~2.9us) matches its
    # PE time (~3.0us), so runs of small slots never build DMA lookahead and
    # the AV phase stalls on its kxn load.  Pairing each small slot with a
    # large one (PE 3.95us vs DMA 3.6us) keeps the prefetch ahead.
    half = BKPC // 2
    order = []
    for i in range(half):
        order += [i, half + i]
    order += list(range(2 * half, BKPC))
    schedule = [schedule[j] for j in order]
    perm = perm.reshape(BKPC, NCORES)[order].reshape(-1)
    return perm, schedule


def _build_nc(schedule):
    from contextlib import ExitStack

    import concourse.bacc as bacc
    import concourse.bass as bass
    import concourse.mybir as mybir
    import concourse.tile as tile

    f16 = mybir.dt.float16
    f32 = mybir.dt.float32
    AF = mybir.ActivationFunctionType
    ALU = mybir.AluOpType

    nc = bacc.Bacc()

    # kxt block t: cols [t*512 + ci*128 + dn] (t-major so slot loads truncate)
    kxt_d = nc.declare_dram_parameter("kxt", [BKPC, 128, NT * C], f16, isOutput=False)
    qg_d = nc.declare_dram_parameter("qg", [128, CT * Bq], f16, isOutput=False)
    kxn_d = nc.declare_dram_parameter("kxn", [BKPC, 128, NT * C], f16, isOutput=False)
    # 0/1 validity columns for all slots: col j*NT + t
    vb_d = nc.declare_dram_parameter("vb", [128, BKPC * NT], f16, isOutput=False)
    # packed output: [b][p][mt][c] — host unpacks to [b, mt*128+p, c]
    out_d = nc.declare_dram_parameter("out", [BKPC, 128, QT * C], f16, isOutput=True)

    with tile.TileContext(nc) as tc, ExitStack() as ctx:
        consts = ctx.enter_context(tc.tile_pool(name="consts", bufs=1))
        work = ctx.enter_context(tc.tile_pool(name="work", bufs=2))
        ps = ctx.enter_context(tc.tile_pool(name="ps", bufs=1, space="PSUM"))

        kxts = {}
        kxns = {}

        def load_slot(j, split=False):
            cj = schedule[j]
            kxt = work.tile([128, NT * C], f16, tag="kxt", bufs=8)
            if split:
                # per-tile loads so the first QK can start after one tile
                for t in range(cj):
                    nc.sync.dma_start(
                        kxt[:, t * C : (t + 1) * C], kxt_d[j, :, t * C : (t + 1) * C]
                    )
            else:
                nc.sync.dma_start(kxt[:, 0 : cj * C], kxt_d[j, :, 0 : cj * C])
            kxn = work.tile([128, NT * C], f16, tag="kxn", bufs=8)
            nc.sync.dma_start(kxn[:, 0 : cj * C], kxn_d[j, :, 0 : cj * C])
            kxts[j] = kxt
            kxns[j] = kxn

        # qg first (small, unblocks the PE), then ramp the slot pipeline;
        # vb is only needed by the first AV phase, so it loads after slot 0
        qg_all = consts.tile([128, CT * Bq], f16)
        nc.sync.dma_start(qg_all[:], qg_d[:, :])
        qgT = [qg_all[:, ci * Bq : (ci + 1) * Bq] for ci in range(CT)]

        # single ACT LUT load for the whole kernel: one dummy Exp up front
        dummy = work.tile([128, 1], f16, tag="dummy")
        nc.vector.memset(dummy[:], 0.0)
        nc.scalar.activation(dummy[:], dummy[:], AF.Exp, scale=0.0)

        # PE warm-up: scratch matmuls during the initial DMA fill ramp the
        # tensor-engine p-state so slot 0 runs at full clock
        warm = work.tile([128, Bq], f16, tag="warm")
        nc.vector.memset(warm[:], 0.0)
        for _ in range(20):
            pw = ps.tile([128, Bq], f32, tag="psa", bufs=5)
            nc.tensor.matmul(pw[:], warm[:, 0:128], warm[:], start=True, stop=True)

        load_slot(0, split=True)
        vb = consts.tile([128, BKPC * NT], f16)
        nc.sync.dma_start(vb[:], vb_d[:, :])
        load_slot(1, split=True)
        for j in range(2, 7):
            load_slot(j)

        for j in range(BKPC):
            cj = schedule[j]
            kxt = kxts.pop(j)
            kxn = kxns.pop(j)
            if j + 7 < BKPC:
                load_slot(j + 7)

            # scores S.T[n, q] per valid n-tile; exp -> pT fp16
            pT = []
            for t in range(cj):
                pa = ps.tile([128, Bq], f32, tag="psa", bufs=5)
                for ci in range(CT):
                    nc.tensor.matmul(
                        pa[:],
                        kxt[:, t * C + ci * 128 : t * C + (ci + 1) * 128],
                        qgT[ci],
                        start=(ci == 0),
                        stop=(ci == CT - 1),
                    )
                pe = work.tile([128, Bq], f16, tag=f"pT{t}", bufs=3)
                nc.scalar.activation(pe[:], pa[:], AF.Exp)
                pT.append(pe)

            # AV + denominator (denominator second: its LDW hides under AV).
            # One 2-col reciprocal after both AV groups frees pd in a single
            # early DVE op (no 900ns normalize mul queued ahead of it), so
            # the next slot's denominator never WAR-stalls on pd.
            osb = work.tile([128, QT * C], f16, tag="osb", bufs=4)
            pd = ps.tile([128, QT], f32, tag="psd", bufs=1)
            rd = work.tile([128, QT], f32, tag="rd", bufs=2)
            pos = []
            for mt in range(QT):
                po = ps.tile([128, C], f32, tag="pso", bufs=2)
                for t in range(cj):
                    lhs = pT[t][:, mt * 128 : (mt + 1) * 128]
                    nc.tensor.matmul(
                        po[:],
                        lhs,
                        kxn[:, t * C : (t + 1) * C],
                        start=(t == 0),
                        stop=(t == cj - 1),
                    )
                    nc.tensor.matmul(
                        pd[:, mt : mt + 1],
                        lhs,
                        vb[:, j * NT + t : j * NT + t + 1],
                        start=(t == 0),
                        stop=(t == cj - 1),
                    )
                pos.append(po)
            nc.vector.reciprocal(rd[:, 0:QT], pd[:, 0:QT])
            # normalize halves on two engines concurrently (Copy is
            # table-free on ACT, so the Exp LUT stays loaded)
            nc.vector.tensor_scalar(
                osb[:, 0:C], pos[0][:], rd[:, 0:1], None, op0=ALU.mult
            )
            nc.scalar.mul(osb[:, C : 2 * C], pos[1][:], rd[:, 1:2])
            if j == BKPC - 1:
                # split the final store so each half leaves as soon as ready
                nc.gpsimd.dma_start(out_d[j, :, 0:C], osb[:, 0:C])
                nc.gpsimd.dma_start(out_d[j, :, C : 2 * C], osb[:, C : 2 * C])
            else:
                nc.gpsimd.dma_start(out_d[j, :, :], osb[:])

    nc.compile()
    return nc


def _prep_host(qx, kx, key_padding_mask, ln_q_g, ln_q_b, ln_k_g, ln_k_b, wq, wk):
    f32 = np.float32

    # ---- q-side: Qg[c, q] fully host-computed (fp32), column-centered ----
    qx32 = np.asarray(qx, f32).reshape(Bq, C)
    m = qx32.mean(-1, keepdims=True)
    v = ((qx32 - m) ** 2).mean(-1, keepdims=True)
    lnq = (qx32 - m) / np.sqrt(v + EPS)
    lnq = lnq * np.asarray(ln_q_g, f32)[None, :] + np.asarray(ln_q_b, f32)[None, :]
    qvec = lnq @ np.asarray(wq, f32).T                      # [Bq, C]
    y = qvec @ np.asarray(wk, f32)                          # [Bq, C]
    G = (y * np.asarray(ln_k_g, f32)[None, :]) * (C ** -0.5)
    G = G - G.mean(axis=1, keepdims=True)                   # center over c
    Qg = np.ascontiguousarray(G.T)                          # [c, q]

    qg_pk = np.zeros((128, CT * Bq), np.float16)
    for ci in range(CT):
        qg_pk[:, ci * Bq : (ci + 1) * Bq] = Qg[ci * 128 : (ci + 1) * 128, :]

    # ---- k-side row stats (host): rk = rsqrt(var + eps), mask folded ----
    kx32 = np.asarray(kx, f32)                              # [Bk, Nk, C]
    km = kx32.mean(-1, keepdims=True)
    kv = ((kx32 - km) ** 2).mean(-1, keepdims=True)
    rk = 1.0 / np.sqrt(kv + EPS)                            # [Bk, Nk, 1]
    mask = np.asarray(key_padding_mask)                     # [Bk, Nk] True=pad
    valid = (~mask).astype(f32)[:, :, None]                 # [Bk, Nk, 1]

    perm, schedule = _schedule_from_mask(mask)

    kxt_full = kx32 * rk * valid                            # [Bk, Nk, C]
    kxn_full = np.asarray(kx, np.float16) * valid.astype(np.float16)

    in_maps = []
    for i in range(NCORES):
        bidx = perm[np.arange(BKPC) * NCORES + i]           # original batch ids
        kxt_s = kxt_full[bidx]                              # [BKPC, Nk, C] f32
        # block t holds all c for n-tile t, c-partition major:
        # kxt[b, p, t*C + ci*128+dn] = kx[b, t*128+dn, ci*128+p]
        kxt_pk = (
            kxt_s.reshape(BKPC, NT, 128, CT, 128)           # [b, t, dn, ci, p]
            .transpose(0, 4, 1, 3, 2)                       # [b, p, t, ci, dn]
            .reshape(BKPC, 128, NT * C)
        )
        kxt_send = np.ascontiguousarray(kxt_pk).astype(np.float16)

        kxn_s = kxn_full[bidx]                              # [BKPC, Nk, C] f16
        kxn_pk = (
            kxn_s.reshape(BKPC, NT, 128, C).transpose(0, 2, 1, 3).reshape(BKPC, 128, NT * C)
        )
        # validity blob: col j*NT + t = valid for keys t*128+p of slot j
        vr = valid[bidx, :, 0].reshape(BKPC, NT, 128).transpose(2, 0, 1)  # [p, b, t]
        vb_pk = np.ascontiguousarray(vr.reshape(128, BKPC * NT)).astype(np.float16)
        in_maps.append(
            dict(
                qg=qg_pk,
                kxt=kxt_send,
                kxn=np.ascontiguousarray(kxn_pk),
                vb=vb_pk,
            )
        )
    return in_maps, perm, schedule


def _get_nc(schedule):
    key = ("nc", tuple(schedule))
    if key not in _cache:
        _cache[key] = _build_nc(schedule)
    return _cache[key]


def kernel(**inputs) -> np.ndarray:
    from concourse.bass_utils import run_bass_kernel_spmd

    in_maps, perm, schedule = _prep_host(**inputs)
    nc = _get_nc(schedule)
    res = run_bass_kernel_spmd(nc, in_maps, list(range(NCORES)))
    full = np.empty((Bq, Bk, C), np.float16)
    for i in range(NCORES):
        o = res.results[i]["out"]  # [BKPC, 128, 2C] packed
        o = o.reshape(BKPC, 128, QT, C).transpose(0, 2, 1, 3).reshape(BKPC, Bq, C)
        bidx = perm[np.arange(BKPC) * NCORES + i]
        full[:, bidx, :] = o.transpose(1, 0, 2)
    return np.ascontiguousarray(full)


# revision 59
# speedup vs baseline: 1.1797x; 1.0051x over previous
"""Trainium2 Bass kernel for nn_Attention_52166672777669 (sparse_attention).

Math (reference):
    q  = LN(qx; g_q, b_q) @ wq.T                        # [256, 512]
    k  = LN(kx; g_k, b_k) @ wk.T                        # [256, 512, 512]
    S[q, kb, n] = (q[q] . k[kb, n]) / sqrt(512)         # masked, softmax over n
    out[q, kb, :] = sum_n P[q, kb, n] * kx[kb, n, :]    # [256, 256, 512]

Algebraic restructuring (exact up to fp rounding):
  S.T[n,q] = <kx[n]*rk_n, Qg[:,q]> with Qg = gk*(wk.T @ q_vec)/sqrt(C),
  column-centered over c (handles the LN mean term exactly since
  sum_c (kx[n,c]-m_n) = 0) and rk_n = rsqrt(var_n + eps).
  All q-side work and the k-side row stats are host-precomputed; rk and
  the padding mask fold into the packed kxt operand (masked columns
  zeroed).  Masked keys then get P = exp(0) = 1, neutralized by zeroed
  kxn rows (numerator) and a 0/1 validity column (denominator).

  Fully-masked 128-key tiles are skipped entirely: batches are sorted by
  valid-tile count and dealt round-robin to the 8 cores, so one static
  per-slot schedule (max count within each rank-8 window) serves all
  cores; skipped tiles contribute exactly zero.

Device inner loop per slot (cj valid n-tiles): 4*cj QK matmuls, cj Exps
on ACT, 2*cj AV + 2*cj denominator matmuls (denominator second so its
redundant LDWEIGHTS hides under the 512-col AV matmul), 2 reciprocal +
2 normalize on DVE.  3 DMAs: loads on sync queue, store on gpsimd queue.

Sharding: Bk split across 8 cores (32 key-batches each). No collectives.
"""

import os
import sys

import numpy as np

for _p in ("/opt/trn_rl_repo",):
    if _p not in sys.path and os.path.isdir(_p):
        sys.path.insert(0, _p)

Bq, Bk, Nk, C = 256, 256, 512, 512
NCORES = 8
BKPC = Bk // NCORES  # key-batch slots per core
EPS = 1e-5
NT = Nk // 128  # 4 n tiles per key batch
CT = C // 128   # 4 c tiles
QT = Bq // 128  # 2 query tiles

_cache = {}


def _schedule_from_mask(mask):
    """Sort batches by valid-tile count asc (small slots first: faster
    pipeline fill), deal round-robin to cores.

    Returns (perm [Bk], schedule [BKPC]) where core i's slot j processes
    original batch perm[j*NCORES + i] using schedule[j] n-tiles."""
    lengths = Nk - np.asarray(mask).sum(axis=1)          # valid keys per batch
    counts = np.ceil(lengths / 128).astype(np.int64)     # needed n-tiles
    perm = np.argsort(counts, kind="stable")
    schedule = [int(counts[perm[j * NCORES + NCORES - 1]]) for j in range(BKPC)]
    # interleave small/large slots: a small slot's DMA (~2.9us) matches its
    # PE time (~3.0us), so runs of small slots never build DMA lookahead and
    # the AV phase stalls on its kxn load.  Pairing each small slot with a
    # large one (PE 3.95us vs DMA 3.6us) keeps the prefetch ahead.
    half = BKPC // 2
    order = []
    for i in range(half):
        order += [i, half + i]
    order += list(range(2 * half, BKPC))
    schedule = [schedule[j] for j in order]
    perm = perm.reshape(BKPC, NCORES)[order].reshape(-1)
    return perm, schedule


def _build_nc(schedule):
    from contextlib import ExitStack

    import concourse.bacc as bacc
    import concourse.bass as bass
    import concourse.mybir as mybir
    import concourse.tile as tile

    f16 = mybir.dt.float16
    f32 = mybir.dt.float32
    AF = mybir.ActivationFunctionType
    ALU = mybir.AluOpType

    nc = bacc.Bacc()

    # kxt block t: cols [t*512 + ci*128 + dn] (t-major so slot loads truncate)
    kxt_d = nc.declare_dram_parameter("kxt", [BKPC, 128, NT * C], f16, isOutput=False)
    qg_d = nc.declare_dram_parameter("qg", [128, CT * Bq], f16, isOutput=False)
    kxn_d = nc.declare_dram_parameter("kxn", [BKPC, 128, NT * C], f16, isOutput=False)
    # 0/1 validity columns for all slots: col j*NT + t
    vb_d = nc.declare_dram_parameter("vb", [128, BKPC * NT], f16, isOutput=False)
    # packed output: [b][p][mt][c] — host unpacks to [b, mt*128+p, c]
    out_d = nc.declare_dram_parameter("out", [BKPC, 128, QT * C], f16, isOutput=True)

    with tile.TileContext(nc) as tc, ExitStack() as ctx:
        consts = ctx.enter_context(tc.tile_pool(name="consts", bufs=1))
        work = ctx.enter_context(tc.tile_pool(name="work", bufs=2))
        ps = ctx.enter_context(tc.tile_pool(name="ps", bufs=1, space="PSUM"))

        kxts = {}
        kxns = {}

        def load_slot(j, split=False):
            cj = schedule[j]
            kxt = work.tile([128, NT * C], f16, tag="kxt", bufs=8)
            if split:
                # per-tile loads so the first QK can start after one tile
                for t in range(cj):
                    nc.sync.dma_start(
                        kxt[:, t * C : (t + 1) * C], kxt_d[j, :, t * C : (t + 1) * C]
                    )
            else:
                nc.sync.dma_start(kxt[:, 0 : cj * C], kxt_d[j, :, 0 : cj * C])
            kxn = work.tile([128, NT * C], f16, tag="kxn", bufs=8)
            nc.sync.dma_start(kxn[:, 0 : cj * C], kxn_d[j, :, 0 : cj * C])
            kxts[j] = kxt
            kxns[j] = kxn

        # qg first (small, unblocks the PE), then ramp the slot pipeline;
        # vb is only needed by the first AV phase, so it loads after slot 0
        qg_all = consts.tile([128, CT * Bq], f16)
        nc.sync.dma_start(qg_all[:], qg_d[:, :])
        qgT = [qg_all[:, ci * Bq : (ci + 1) * Bq] for ci in range(CT)]

        # single ACT LUT load for the whole kernel: one dummy Exp up front
        dummy = work.tile([128, 1], f16, tag="dummy")
        nc.vector.memset(dummy[:], 0.0)
        nc.scalar.activation(dummy[:], dummy[:], AF.Exp, scale=0.0)

        # PE warm-up: scratch matmuls during the initial DMA fill ramp the
        # tensor-engine p-state so slot 0 runs at full clock
        warm = work.tile([128, Bq], f16, tag="warm")
        nc.vector.memset(warm[:], 0.0)
        for _ in range(20):
            pw = ps.tile([128, Bq], f32, tag="psa", bufs=4)
            nc.tensor.matmul(pw[:], warm[:, 0:128], warm[:], start=True, stop=True)

        load_slot(0, split=True)
        vb = consts.tile([128, BKPC * NT], f16)
        nc.sync.dma_start(vb[:], vb_d[:, :])
        load_slot(1, split=True)
        for j in range(2, 7):
            load_slot(j)

        for j in range(BKPC):
            cj = schedule[j]
            kxt = kxts.pop(j)
            kxn = kxns.pop(j)
            if j + 7 < BKPC:
                load_slot(j + 7)

            # scores S.T[n, q] per valid n-tile; exp -> pT fp16
            pT = []
            for t in range(cj):
                pa = ps.tile([128, Bq], f32, tag="psa", bufs=4)
                for ci in range(CT):
                    nc.tensor.matmul(
                        pa[:],
                        kxt[:, t * C + ci * 128 : t * C + (ci + 1) * 128],
                        qgT[ci],
                        start=(ci == 0),
                        stop=(ci == CT - 1),
                    )
                pe = work.tile([128, Bq], f16, tag=f"pT{t}", bufs=3)
                nc.scalar.activation(pe[:], pa[:], AF.Exp)
                pT.append(pe)

            # AV + denominator (denominator second: its LDW hides under AV).
            # One 2-col reciprocal after both AV groups frees pd in a single
            # early DVE op (no 900ns normalize mul queued ahead of it), so
            # the next slot's denominator never WAR-stalls on pd.
            osb = work.tile([128, QT * C], f16, tag="osb", bufs=4)
            pd = ps.tile([128, QT], f32, tag="psd", bufs=1)
            rd = work.tile([128, QT], f32, tag="rd", bufs=2)
            pos = []
            for mt in range(QT):
                po = ps.tile([128, C], f32, tag="pso", bufs=3)
                for t in range(cj):
                    lhs = pT[t][:, mt * 128 : (mt + 1) * 128]
                    nc.tensor.matmul(
                        po[:],
                        lhs,
                        kxn[:, t * C : (t + 1) * C],
                        start=(t == 0),
                        stop=(t == cj - 1),
                    )
                    nc.tensor.matmul(
                        pd[:, mt : mt + 1],
                        lhs,
                        vb[:, j * NT + t : j * NT + t + 1],
                        start=(t == 0),
                        stop=(t == cj - 1),
                    )
                pos.append(po)
            nc.vector.reciprocal(rd[:, 0:QT], pd[:, 0:QT])
            # normalize halves on two engines concurrently (Copy is
            # table-free on ACT, so the Exp LUT stays loaded)
            nc.vector.tensor_scalar(
                osb[:, 0:C], pos[0][:], rd[:, 0:1], None, op0=ALU.mult
            )
            nc.scalar.mul(osb[:, C : 2 * C], pos[1][:], rd[:, 1:2])
            if j == BKPC - 1:
                # split the final store so each half leaves as soon as ready
                nc.gpsimd.dma_start(out_d[j, :, 0:C], osb[:, 0:C])
                nc.gpsimd.dma_start(out_d[j, :, C : 2 * C], osb[:, C : 2 * C])
            else:
                nc.gpsimd.dma_start(out_d[j, :, :], osb[:])

    nc.compile()
    return nc


def _prep_host(qx, kx, key_padding_mask, ln_q_g, ln_q_b, ln_k_g, ln_k_b, wq, wk):
    f32 = np.float32

    # ---- q-side: Qg[c, q] fully host-computed (fp32), column-centered ----
    qx32 = np.asarray(qx, f32).reshape(Bq, C)
    m = qx32.mean(-1, keepdims=True)
    v = ((qx32 - m) ** 2).mean(-1, keepdims=True)
    lnq = (qx32 - m) / np.sqrt(v + EPS)
    lnq = lnq * np.asarray(ln_q_g, f32)[None, :] + np.asarray(ln_q_b, f32)[None, :]
    qvec = lnq @ np.asarray(wq, f32).T                      # [Bq, C]
    y = qvec @ np.asarray(wk, f32)                          # [Bq, C]
    G = (y * np.asarray(ln_k_g, f32)[None, :]) * (C ** -0.5)
    G = G - G.mean(axis=1, keepdims=True)                   # center over c
    Qg = np.ascontiguousarray(G.T)                          # [c, q]

    qg_pk = np.zeros((128, CT * Bq), np.float16)
    for ci in range(CT):
        qg_pk[:, ci * Bq : (ci + 1) * Bq] = Qg[ci * 128 : (ci + 1) * 128, :]

    # ---- k-side row stats (host): rk = rsqrt(var + eps), mask folded ----
    kx32 = np.asarray(kx, f32)                              # [Bk, Nk, C]
    km = kx32.mean(-1, keepdims=True)
    kv = ((kx32 - km) ** 2).mean(-1, keepdims=True)
    rk = 1.0 / np.sqrt(kv + EPS)                            # [Bk, Nk, 1]
    mask = np.asarray(key_padding_mask)                     # [Bk, Nk] True=pad
    valid = (~mask).astype(f32)[:, :, None]                 # [Bk, Nk, 1]

    perm, schedule = _schedule_from_mask(mask)

    kxt_full = kx32 * rk * valid                            # [Bk, Nk, C]
    kxn_full = np.asarray(kx, np.float16) * valid.astype(np.float16)

    in_maps = []
    for i in range(NCORES):
        bidx = perm[np.arange(BKPC) * NCORES + i]           # original batch ids
        kxt_s = kxt_full[bidx]                              # [BKPC, Nk, C] f32
        # block t holds all c for n-tile t, c-partition major:
        # kxt[b, p, t*C + ci*128+dn] = kx[b, t*128+dn, ci*128+p]
        kxt_pk = (
            kxt_s.reshape(BKPC, NT, 128, CT, 128)           # [b, t, dn, ci, p]
            .transpose(0, 4, 1, 3, 2)                       # [b, p, t, ci, dn]
            .reshape(BKPC, 128, NT * C)
        )
        kxt_send = np.ascontiguousarray(kxt_pk).astype(np.float16)

        kxn_s = kxn_full[bidx]                              # [BKPC, Nk, C] f16
        kxn_pk = (
            kxn_s.reshape(BKPC, NT, 128, C).transpose(0, 2, 1, 3).reshape(BKPC, 128, NT * C)
        )
        # validity blob: col j*NT + t = valid for keys t*128+p of slot j
        vr = valid[bidx, :, 0].reshape(BKPC, NT, 128).transpose(2, 0, 1)  # [p, b, t]
        vb_pk = np.ascontiguousarray(vr.reshape(128, BKPC * NT)).astype(np.float16)
        in_maps.append(
            dict(
                qg=qg_pk,
                kxt=kxt_send,
                kxn=np.ascontiguousarray(kxn_pk),
                vb=vb_pk,
            )
        )
    return in_maps, perm, schedule


def _get_nc(schedule):
    key = ("nc", tuple(schedule))
    if key not in _cache:
        _cache[key] = _build_nc(schedule)
    return _cache[key]


def kernel(**inputs) -> np.ndarray:
    from concourse.bass_utils import run_bass_kernel_spmd

    in_maps, perm, schedule = _prep_host(**inputs)
    nc = _get_nc(schedule)
    res = run_bass_kernel_spmd(nc, in_maps, list(range(NCORES)))
    full = np.empty((Bq, Bk, C), np.float16)
    for i in range(NCORES):
        o = res.results[i]["out"]  # [BKPC, 128, 2C] packed
        o = o.reshape(BKPC, 128, QT, C).transpose(0, 2, 1, 3).reshape(BKPC, Bq, C)
        bidx = perm[np.arange(BKPC) * NCORES + i]
        full[:, bidx, :] = o.transpose(1, 0, 2)
    return np.ascontiguousarray(full)
